# revision 1
# baseline (speedup 1.0000x reference)
"""AttentivePoolingNetwork Trainium2 kernel.

Data-parallel over batch across 8 NeuronCores (64 batch elements each).
Per batch element, fully fused on-chip:
  gather bf16 emb rows -> PE-transpose to [E, L] -> conv1d(k=3) as shifted
  matmuls (bias folded in via ones-row) -> QT/AT [tokens, F] -> transposes
  -> H = U^T Q -> G = H^T A -> row/col maxes -> exp(tanh(max)) weights
  (softmax denominators cancel in the final cosine similarity) -> pooled
  rQ/rA via tiny matmuls -> cosine similarity per element.
"""

import os
import numpy as np
import ml_dtypes

import concourse.bacc as bacc
import concourse.bass as bass
import concourse.tile as tile
import concourse.mybir as mybir
from concourse import bass_utils
from concourse.masks import make_identity

BF16 = mybir.dt.bfloat16
F32 = mybir.dt.float32
I32 = mybir.dt.int32
AX = mybir.AxisListType.X
AF = mybir.ActivationFunctionType

B, QL, AL = 512, 128, 512
V1, E, F = 50001, 300, 400
NCORES = 8
BL = int(os.environ.get("KBL", B // NCORES))  # batch elems per core
ABL = set(os.environ.get("ABL", "").split(","))  # ablation flags (timing expts)
EP = 320   # emb width padded (300 -> 320), bf16 rows = 640B
FP = 512   # feature width padded (400 -> 512)


def build_kernel(nc):
    emb = nc.dram_tensor("emb", [V1, EP], BF16, kind="ExternalInput").ap()
    qidx = nc.dram_tensor("qidx", [128, BL], I32, kind="ExternalInput").ap()
    aidx = nc.dram_tensor("aidx", [128, 4 * BL], I32, kind="ExternalInput").ap()
    wc0 = nc.dram_tensor("wc0", [128, 1200], BF16, kind="ExternalInput").ap()
    wc1 = nc.dram_tensor("wc1", [128, 1200], BF16, kind="ExternalInput").ap()
    wc2 = nc.dram_tensor("wc2", [65, 1200], BF16, kind="ExternalInput").ap()
    u_s = nc.dram_tensor("u_s", [128, 2048], BF16, kind="ExternalInput").ap()
    out_d = nc.dram_tensor("out", [BL], F32, kind="ExternalOutput").ap()

    with tile.TileContext(nc) as tc:
        def bufs(name, dflt):
            return int(os.environ.get("BUFS_" + name, dflt))
        with (
            tc.tile_pool(name="const", bufs=1) as cpool,
            tc.tile_pool(name="xg", bufs=bufs("xg", 2)) as xgp,
            tc.tile_pool(name="xt", bufs=bufs("xt", 2)) as xtp,
            tc.tile_pool(name="cs", bufs=bufs("cs", 3)) as csp,
            tc.tile_pool(name="as_", bufs=bufs("as_", 10)) as asp,
            tc.tile_pool(name="qp", bufs=bufs("qp", 2)) as qpp,
            tc.tile_pool(name="ag", bufs=bufs("ag", 8)) as agp,
            tc.tile_pool(name="hg", bufs=bufs("hg", 2)) as hgp,
            tc.tile_pool(name="sm", bufs=bufs("sm", 3)) as smp,
            tc.tile_pool(name="pconv", bufs=bufs("pconv", 2), space="PSUM") as pcv,
            tc.tile_pool(name="ptr", bufs=bufs("ptr", 4), space="PSUM") as ptr,
            tc.tile_pool(name="pg", bufs=bufs("pg", 1), space="PSUM") as pgp,
            tc.tile_pool(name="pr", bufs=bufs("pr", 1), space="PSUM") as prp,
        ):
            idn = cpool.tile([128, 128], BF16)
            make_identity(nc, idn[:])
            qi = cpool.tile([128, BL], I32)
            nc.sync.dma_start(qi[:], qidx)
            ai = cpool.tile([128, 4 * BL], I32)
            nc.sync.dma_start(ai[:], aidx)
            w0 = cpool.tile([128, 1200], BF16)
            nc.sync.dma_start(w0[:], wc0)
            w1 = cpool.tile([128, 1200], BF16)
            nc.sync.dma_start(w1[:], wc1)
            w2 = cpool.tile([65, 1200], BF16)
            nc.sync.dma_start(w2[:], wc2)
            uu = cpool.tile([128, 2048], BF16)
            nc.sync.dma_start(uu[:], u_s)
            dot_acc = cpool.tile([1, BL], F32)
            q2_acc = cpool.tile([1, BL], F32)
            a2_acc = cpool.tile([1, BL], F32)

            def emit_tail(b, g_s, qt_s, at_s, eq):
                pgt2 = ptr.tile([128, 512], BF16, tag="ptr")
                ma = smp.tile([128, 4], F32, tag="ma")
                for m in range(4):
                    nc.tensor.transpose(out=pgt2[:, 128 * m:128 * m + 128],
                                        in_=g_s[:, 128 * m:128 * m + 128],
                                        identity=idn[:])
                for m in range(4):
                    nc.vector.reduce_max(out=ma[:, m:m + 1],
                                         in_=pgt2[:, 128 * m:128 * m + 128], axis=AX)
                ta = smp.tile([128, 4], F32, tag="ta")
                ea = smp.tile([128, 4], BF16, tag="ea")
                nc.scalar.activation(out=ta[:], in_=ma[:], func=AF.Tanh)
                nc.scalar.activation(out=ea[:], in_=ta[:], func=AF.Exp)
                prq = prp.tile([1, 400], F32, tag="pr")
                nc.tensor.matmul(out=prq[:], lhsT=eq[:], rhs=qt_s[:, 0:400],
                                 start=True, stop=True)
                pra = prp.tile([1, 400], F32, tag="pr")
                for m in range(4):
                    nc.tensor.matmul(out=pra[:], lhsT=ea[:, m:m + 1],
                                     rhs=at_s[m][:, 0:400],
                                     start=(m == 0), stop=(m == 3))
                rq_s = smp.tile([1, 400], F32, tag="rqs")
                nc.any.tensor_copy(out=rq_s[:], in_=prq[:])
                prod = smp.tile([1, 400], F32, tag="prod")
                nc.vector.tensor_mul(out=prod[:], in0=rq_s[:], in1=pra[:])
                nc.vector.reduce_sum(out=dot_acc[0:1, b:b + 1], in_=prod[:], axis=AX)
                scr1 = smp.tile([1, 400], F32, tag="scr1")
                nc.scalar.activation(out=scr1[:], in_=prq[:], func=AF.Square,
                                     accum_out=q2_acc[0:1, b:b + 1])
                scr2 = smp.tile([1, 400], F32, tag="scr2")
                nc.scalar.activation(out=scr2[:], in_=pra[:], func=AF.Square,
                                     accum_out=a2_acc[0:1, b:b + 1])

            pending = []
            PIPE = int(os.environ.get("PIPE", 1))
            for b in range(BL):
                # gather: block 0 = question, blocks 1..4 = answer chunks
                xg = xgp.tile([128, 5 * EP], BF16, tag="xg")
                if "gather" in ABL:
                    nc.vector.memset(xg[:], 0.0)
                if "gather" not in ABL:
                  nc.gpsimd.indirect_dma_start(
                    out=xg[:, 0:EP], out_offset=None, in_=emb,
                    in_offset=bass.IndirectOffsetOnAxis(ap=qi[:, b:b + 1], axis=0))
                  for m in range(4):
                    nc.gpsimd.indirect_dma_start(
                        out=xg[:, (m + 1) * EP:(m + 2) * EP], out_offset=None,
                        in_=emb,
                        in_offset=bass.IndirectOffsetOnAxis(
                            ap=ai[:, 4 * b + m:4 * b + m + 1], axis=0))

                # transpose gathered [tokens, E] -> xT chunks [e, 643]:
                # col 0 = pad, 1:129 = question, 129 = pad, 130:642 = answer,
                # 642 = pad. Pads make every shifted conv window a full 128
                # cols so matmul outputs always span partitions 0:128.
                xt1 = xtp.tile([128, 643], BF16, tag="xt1")
                xt2 = xtp.tile([128, 643], BF16, tag="xt2")
                xt3 = xtp.tile([65, 643], BF16, tag="xt3")
                for xt in (xt1, xt2, xt3):
                    nc.any.memset(xt[:, 0:1], 0.0)
                    nc.any.memset(xt[:, 129:130], 0.0)
                    nc.any.memset(xt[:, 642:643], 0.0)
                nc.any.memset(xt3[64:65, :], 1.0)

                def emit_xt(tb):
                    px = ptr.tile([128, 384], BF16, tag="ptr")
                    src = xg[:, tb * EP:(tb + 1) * EP]
                    nc.tensor.transpose(out=px[0:128, 0:128], in_=src[:, 0:128],
                                        identity=idn[:])
                    nc.tensor.transpose(out=px[0:128, 128:256], in_=src[:, 128:256],
                                        identity=idn[:])
                    nc.tensor.transpose(out=px[0:64, 256:384], in_=src[:, 256:320],
                                        identity=idn[:])
                    c0 = 1 + 128 * tb if tb == 0 else 130 + 128 * (tb - 1)
                    nc.any.tensor_copy(out=xt1[:, c0:c0 + 128], in_=px[0:128, 0:128])
                    nc.any.tensor_copy(out=xt2[:, c0:c0 + 128], in_=px[0:128, 128:256])
                    nc.any.tensor_copy(out=xt3[0:64, c0:c0 + 128], in_=px[0:64, 256:384])

                # conv1d as shifted matmuls: out[l, f] = sum_{e,k} x[l+k-1, e] w_k[e, f]
                def conv_block(dst_ps, seg0):
                    first = True
                    for ec, xt, csz in ((0, xt1, 128), (1, xt2, 128), (2, xt3, 65)):
                        w = (w0, w1, w2)[ec]
                        for k in (0, 1, 2):
                            c = seg0 + k - 1
                            nc.tensor.matmul(
                                out=dst_ps[0:128, 0:400],
                                lhsT=xt[0:csz, c:c + 128],
                                rhs=w[0:csz, 400 * k:400 * k + 400],
                                start=first, stop=(ec == 2 and k == 2))
                            first = False

                for tb in range(5) if "xt" not in ABL else []:
                    emit_xt(tb)
                qt_s = csp.tile([128, FP], BF16, tag="qt")
                pq = pcv.tile([128, 400], F32, tag="pconv")
                if "conv" not in ABL:
                    conv_block(pq, 1)
                nc.any.tensor_copy(out=qt_s[:, 0:400], in_=pq[:])
                nc.any.memset(qt_s[:, 400:512], 0.0)
                at_s = []
                for m in range(4):
                    pa = pcv.tile([128, 400], F32, tag="pconv")
                    if "conv" not in ABL:
                        conv_block(pa, 130 + 128 * m)
                    t = asp.tile([128, FP], BF16, tag="at")
                    nc.any.tensor_copy(out=t[:, 0:400], in_=pa[:])
                    nc.any.memset(t[:, 400:512], 0.0)
                    at_s.append(t)

                # transposes: QT -> Q_pack [f-chunk, q], AT -> A_g[j] [g-chunk, a]
                q_pack = qpp.tile([128, FP], BF16, tag="qpack")
                pt = ptr.tile([128, 512], BF16, tag="ptr")
                for j in range(4) if "qat" not in ABL else []:
                    nc.tensor.transpose(out=pt[:, 128 * j:128 * j + 128],
                                        in_=qt_s[:, 128 * j:128 * j + 128],
                                        identity=idn[:])
                nc.any.tensor_copy(out=q_pack[:], in_=pt[:])
                def emit_at2(m):
                    pt2 = ptr.tile([128, 512], BF16, tag="ptr")
                    for j in range(4):
                        nc.tensor.transpose(out=pt2[:, 128 * j:128 * j + 128],
                                            in_=at_s[m][:, 128 * j:128 * j + 128],
                                            identity=idn[:])
                    agt = agp.tile([128, FP], BF16, tag="ag")
                    nc.any.tensor_copy(out=agt[:], in_=pt2[:])
                    a_t2.append(agt)

                a_t2 = []
                for m in range(3) if "qat" not in ABL else []:
                    emit_at2(m)

                # H[g, q] = sum_f U[f, g] Q[f, q]  (emitted before the last
                # A-transpose so its copy drains under H's matmuls)
                ph = pgp.tile([128, 512], F32, tag="pg")
                for i in range(4) if "ug" not in ABL else []:
                    for j in range(4):
                        nc.tensor.matmul(
                            out=ph[:, 128 * j:128 * j + 128],
                            lhsT=uu[:, 512 * i + 128 * j:512 * i + 128 * j + 128],
                            rhs=q_pack[:, 128 * i:128 * i + 128],
                            start=(i == 0), stop=(i == 3))
                if "qat" not in ABL:
                    emit_at2(3)
                h_s = hgp.tile([128, 512], BF16, tag="hs")
                nc.any.tensor_copy(out=h_s[:], in_=ph[:])

                # G[q, a] = sum_g H[g, q] A[g, a]
                pgt = pgp.tile([128, 512], F32, tag="pg")
                for m in range(4) if "ug" not in ABL else []:
                    for j in range(4):
                        nc.tensor.matmul(
                            out=pgt[:, 128 * m:128 * m + 128],
                            lhsT=h_s[:, 128 * j:128 * j + 128],
                            rhs=a_t2[m][:, 128 * j:128 * j + 128],
                            start=(j == 0), stop=(j == 3))
                mq = smp.tile([128, 1], F32, tag="mq")
                nc.vector.reduce_max(out=mq[:], in_=pgt[:], axis=AX)
                g_s = hgp.tile([128, 512], BF16, tag="gs")
                nc.any.tensor_copy(out=g_s[:], in_=pgt[:])
                tq = smp.tile([128, 1], F32, tag="tq")
                eq = smp.tile([128, 1], BF16, tag="eq")
                nc.scalar.activation(out=tq[:], in_=mq[:], func=AF.Tanh)
                nc.scalar.activation(out=eq[:], in_=tq[:], func=AF.Exp)

                # defer the tail (G^T maxes + pooling) one iteration so its
                # cross-engine operands are ready when the in-order PE queue
                # reaches it (software pipelining of the PE stall).
                pending.append((b, g_s, qt_s, at_s, eq))
                if len(pending) > PIPE:
                    emit_tail(*pending.pop(0))

            for p in pending:
                emit_tail(*p)

            # cosine similarity finalize on [1, BL] vectors
            den = cpool.tile([1, BL], F32)
            nc.vector.tensor_mul(out=den[:], in0=q2_acc[:], in1=a2_acc[:])
            sq = cpool.tile([1, BL], F32)
            nc.scalar.activation(out=sq[:], in_=den[:], func=AF.Sqrt)
            inv = cpool.tile([1, BL], F32)
            nc.vector.reciprocal(out=inv[:], in_=sq[:])
            res = cpool.tile([1, BL], F32)
            nc.vector.tensor_mul(out=res[:], in0=dot_acc[:], in1=inv[:])
            nc.sync.dma_start(out_d.rearrange("(a b) -> a b", a=1), res[:])
    return nc


_BUILT = {}


def get_built():
    if "nc" not in _BUILT:
        nc = bacc.Bacc("TRN2", target_bir_lowering=False, debug=False,
                       num_devices=NCORES)
        build_kernel(nc)
        nc.compile()
        _BUILT["nc"] = nc
    return _BUILT["nc"]


def prep_inputs(question, answer, emb_table, conv_w, conv_b, U):
    bf = ml_dtypes.bfloat16
    emb_pad = np.zeros((V1, EP), dtype=bf)
    emb_pad[:, :E] = emb_table.astype(bf)

    wt = np.ascontiguousarray(conv_w.astype(np.float32).transpose(1, 0, 2))  # [E, F, K]
    wc0 = np.zeros((128, 1200), dtype=bf)
    wc1 = np.zeros((128, 1200), dtype=bf)
    wc2 = np.zeros((65, 1200), dtype=bf)
    for k in range(3):
        wc0[:, 400 * k:400 * k + 400] = wt[0:128, :, k].astype(bf)
        wc1[:, 400 * k:400 * k + 400] = wt[128:256, :, k].astype(bf)
        wc2[0:44, 400 * k:400 * k + 400] = wt[256:300, :, k].astype(bf)
    wc2[64, 400:800] = conv_b.astype(bf)  # bias row, k=1 block only

    u_pad = np.zeros((512, 512), dtype=np.float32)
    u_pad[:400, :400] = U.astype(np.float32)
    u_sh = np.zeros((128, 2048), dtype=bf)
    for i in range(4):
        u_sh[:, 512 * i:512 * i + 512] = u_pad[128 * i:128 * i + 128, :].astype(bf)

    qi = question.astype(np.int32)  # [B, 128]
    ai = answer.astype(np.int32)    # [B, 512]
    in_maps = []
    for c in range(NCORES):
        qs = qi[c * (B // NCORES):(c + 1) * (B // NCORES)][:BL]     # [BL, 128]
        as_ = ai[c * (B // NCORES):(c + 1) * (B // NCORES)][:BL]    # [BL, 512]
        qidx = np.ascontiguousarray(qs.T)                           # [128, BL]
        aidx = np.ascontiguousarray(
            as_.reshape(BL, 4, 128).transpose(2, 0, 1).reshape(128, 4 * BL))
        in_maps.append({
            "emb": emb_pad, "qidx": qidx, "aidx": aidx,
            "wc0": wc0, "wc1": wc1, "wc2": wc2, "u_s": u_sh,
        })
    return in_maps


def kernel(question, answer, emb_table, conv_w, conv_b, U):
    question = np.asarray(question)
    answer = np.asarray(answer)
    emb_table = np.asarray(emb_table, dtype=np.float32)
    conv_w = np.asarray(conv_w, dtype=np.float32)
    conv_b = np.asarray(conv_b, dtype=np.float32)
    U = np.asarray(U, dtype=np.float32)

    nc = get_built()
    in_maps = prep_inputs(question, answer, emb_table, conv_w, conv_b, U)
    res = bass_utils.run_bass_kernel_spmd(nc, in_maps, core_ids=list(range(NCORES)))
    out = np.concatenate([np.asarray(res.results[c]["out"]).reshape(-1)
                          for c in range(NCORES)])
    return out.astype(np.float32)



# revision 5
# speedup vs baseline: 1.7411x; 1.7411x over previous
"""AttentivePoolingNetwork Trainium2 kernel, v2.

Data-parallel over batch across 8 NeuronCores (64 batch elements each).

Key idea: the conv1d is a per-token linear map of the embedding, so the
per-vocab contributions C_k[v] = W_k @ emb[v] (k = 3 taps, bias folded into
C_1) are precomputed host-side into fp16 tables. On device the conv output
y[l] = C_0[t(l-1)] + C_1[t(l)] + C_2[t(l+1)] is produced by three indirect
DMA gathers with CCE-add accumulation — no PE conv at all. A zero sentinel
row (index V) handles sequence edges.

Per batch element, on chip:
  y tiles [tokens, 400] fp16 (1 question block + 4 answer blocks)
  -> PE transposes + x4-scaled fp8 copies -> q8/at2 [f, tokens] fp8
  -> H = U^T Q, G = H^T A, G^T = A^T H as fp8 DoubleRow matmuls (2 k-tiles
     per instruction at 0.5 cycles/row)
  -> row maxes of G and G^T -> tanh -> exp -> pooling weights (softmax
     denominators cancel in the cosine similarity)
  -> pooled rQ/rA via N=1 matmuls (lhsT = y tiles, rhs = weight vectors)
  -> batched cosine-similarity finalize over all 64 elements.
"""

import numpy as np
import ml_dtypes

import concourse.bacc as bacc
import concourse.bass as bass
import concourse.tile as tile
import concourse.mybir as mybir
from concourse import bass_utils
from concourse.masks import make_identity

FP16 = mybir.dt.float16
FP8 = mybir.dt.float8e4
BF16 = mybir.dt.bfloat16
F32 = mybir.dt.float32
I32 = mybir.dt.int32
AX = mybir.AxisListType.X
AF = mybir.ActivationFunctionType
DR = mybir.MatmulPerfMode.DoubleRow
ADD = mybir.AluOpType.add
BYP = mybir.AluOpType.bypass

B, QL, AL = 512, 128, 512
V1, E, F = 50001, 300, 400
VROWS = V1 + 1          # + zero sentinel row for sequence edges
NCORES = 8
BL = B // NCORES        # 64 batch elements per core
M = 8                   # elements per gather group
NG = BL // M
NBLK = 5                # token blocks per element: 1 question + 4 answer

SC_Y8 = 4.0             # y -> fp8 scale on the G path
SC_H8 = 1.0 / 128.0     # H(dev) -> fp8 scale
SC_U8 = 256.0           # U -> fp8 scale (host)
SC_TANH = 1.0 / (SC_Y8 * SC_Y8 * SC_U8 * SC_H8)  # undo scales before tanh


def build_kernel(nc):
    cc0 = nc.dram_tensor("cc0", [VROWS, F], FP16, kind="ExternalInput").ap()
    cc1 = nc.dram_tensor("cc1", [VROWS, F], FP16, kind="ExternalInput").ap()
    cc2 = nc.dram_tensor("cc2", [VROWS, F], FP16, kind="ExternalInput").ap()
    gidx = nc.dram_tensor("gidx", [128, 3 * BL * NBLK], I32,
                          kind="ExternalInput").ap()
    uu8 = nc.dram_tensor("uu8", [128, 2048], FP8, kind="ExternalInput").ap()
    out_d = nc.dram_tensor("out", [BL], F32, kind="ExternalOutput").ap()
    ccs = (cc0, cc1, cc2)

    with tile.TileContext(nc) as tc:
        with (
            tc.tile_pool(name="const", bufs=1) as cpool,
            tc.tile_pool(name="yg", bufs=2) as ygp,
            tc.tile_pool(name="q8", bufs=2) as q8p,
            tc.tile_pool(name="at", bufs=8) as atp,
            tc.tile_pool(name="h8", bufs=2) as h8p,
            tc.tile_pool(name="sm", bufs=3) as smp,
            tc.tile_pool(name="ptr", bufs=2, space="PSUM") as ptr,
            tc.tile_pool(name="ph", bufs=2, space="PSUM") as php,
            tc.tile_pool(name="pg", bufs=1, space="PSUM") as pgp,
            tc.tile_pool(name="pt2", bufs=1, space="PSUM") as pt2p,
            tc.tile_pool(name="pr", bufs=1, space="PSUM") as prp,
            tc.tile_pool(name="pf", bufs=1, space="PSUM") as pfp,
        ):
            idn = cpool.tile([128, 128], FP16)
            make_identity(nc, idn[:])
            ix = cpool.tile([128, 3 * BL * NBLK], I32)
            nc.sync.dma_start(ix[:], gidx)
            uu = cpool.tile([128, 2048], FP8)
            nc.sync.dma_start(uu[:], uu8)
            ones = cpool.tile([128, 1], BF16)
            nc.vector.memset(ones[:], 1.0)
            # rq at cols [4b : 4b+4], ra at cols [256 + 4b : +4]; zero so the
            # partition tails of the 16-row f-chunk-3 columns stay zero.
            rball = cpool.tile([128, 512], F32)
            nc.vector.memset(rball[:], 0.0)

            YW = NBLK * F          # 2000 cols per element in a y group tile
            GW = M * YW            # 16000 cols per group

            def emit_tail(b, yq_off, ya_offs, ygrp, e_all):
                pr = prp.tile([128, 8], F32, tag="pr")
                for c in range(4):
                    w = 128 if c < 3 else 16
                    nc.tensor.matmul(
                        out=pr[0:w, c:c + 1],
                        lhsT=ygrp[:, yq_off + 128 * c: yq_off + 128 * c + w],
                        rhs=e_all[:, 0:1], start=True, stop=True)
                for c in range(4):
                    w = 128 if c < 3 else 16
                    for m in range(4):
                        nc.tensor.matmul(
                            out=pr[0:w, 4 + c:5 + c],
                            lhsT=ygrp[:, ya_offs[m] + 128 * c:
                                      ya_offs[m] + 128 * c + w],
                            rhs=e_all[:, 1 + m:2 + m],
                            start=(m == 0), stop=(m == 3))
                nc.gpsimd.tensor_copy(out=rball[:, 4 * b:4 * b + 3],
                                      in_=pr[:, 0:3])
                nc.gpsimd.tensor_copy(out=rball[0:16, 4 * b + 3:4 * b + 4],
                                      in_=pr[0:16, 3:4])
                nc.gpsimd.tensor_copy(out=rball[:, 256 + 4 * b:256 + 4 * b + 3],
                                      in_=pr[:, 4:7])
                nc.gpsimd.tensor_copy(out=rball[0:16, 256 + 4 * b + 3:
                                                256 + 4 * b + 4],
                                      in_=pr[0:16, 7:8])

            pending = []
            for g in range(NG):
                ygrp = ygp.tile([128, GW + 128], FP16, tag="yg")
                # keep the over-read tail of the chunk-3 transposes finite
                nc.vector.memset(ygrp[:, GW:GW + 128], 0.0)
                out_ap = ygrp[:, 0:GW].rearrange("p (n f) -> p n f", f=F)
                for k in (1, 0, 2):
                    sec = k * BL * NBLK + g * M * NBLK
                    nc.gpsimd.indirect_dma_start(
                        out=out_ap, out_offset=None, in_=ccs[k],
                        in_offset=bass.IndirectOffsetOnAxis(
                            ap=ix[:, sec:sec + M * NBLK], axis=0),
                        compute_op=(BYP if k == 1 else ADD))

                for e in range(M):
                    b = g * M + e
                    yq_off = e * YW
                    ya_offs = [e * YW + (1 + m) * F for m in range(4)]

                    # transpose question block -> q8 [f, q] fp8 (x4)
                    tpq = ptr.tile([128, 512], FP16, tag="tp")
                    for c in range(4):
                        nc.tensor.transpose(
                            out=tpq[:, 128 * c:128 * c + 128],
                            in_=ygrp[:, yq_off + 128 * c:yq_off + 128 * c + 128],
                            identity=idn[:])
                    q8t = q8p.tile([128, 512], FP8, tag="q8")
                    nc.scalar.activation(out=q8t[:], in_=tpq[:], func=AF.Copy,
                                         scale=SC_Y8)

                    # transpose answer blocks -> at2[m] [g, a] fp8 (x4)
                    at2 = []
                    for m in range(4):
                        tpa = ptr.tile([128, 512], FP16, tag="tp")
                        for c in range(4):
                            nc.tensor.transpose(
                                out=tpa[:, 128 * c:128 * c + 128],
                                in_=ygrp[:, ya_offs[m] + 128 * c:
                                         ya_offs[m] + 128 * c + 128],
                                identity=idn[:])
                        at = atp.tile([128, 512], FP8, tag="at")
                        if m == 0:
                            nc.vector.tensor_scalar_mul(out=at[:], in0=tpa[:],
                                                        scalar1=SC_Y8)
                        elif m in (1, 2):
                            nc.gpsimd.tensor_scalar_mul(out=at[:], in0=tpa[:],
                                                        scalar1=SC_Y8)
                        else:
                            nc.scalar.activation(out=at[:], in_=tpa[:],
                                                 func=AF.Copy, scale=SC_Y8)
                        at2.append(at)

                    # H[g, q] = sum_f U8[f, g] Q8[f, q]  (DoubleRow pairs)
                    ph = php.tile([128, 512], F32, tag="ph")
                    for j in range(4):
                        for ip in range(2):
                            nc.tensor.matmul(
                                out=ph[:, 128 * j:128 * j + 128],
                                lhsT=uu[:, 256 * (2 * j + ip):
                                        256 * (2 * j + ip) + 256].rearrange(
                                    "p (t m) -> p t m", t=2),
                                rhs=q8t[:, 256 * ip:256 * ip + 256].rearrange(
                                    "p (t n) -> p t n", t=2),
                                start=(ip == 0), stop=(ip == 1), perf_mode=DR)
                    h8t = h8p.tile([128, 512], FP8, tag="h8")
                    nc.scalar.activation(out=h8t[:], in_=ph[:], func=AF.Copy,
                                         scale=SC_H8)

                    # G[q, a] = sum_g H[g, q] A[g, a]; G^T likewise
                    pg = pgp.tile([128, 512], F32, tag="pg")
                    pgt = pt2p.tile([128, 512], F32, tag="pgt")
                    for m in range(4):
                        for jp in range(2):
                            h_pair = h8t[:, 256 * jp:256 * jp + 256].rearrange(
                                "p (t n) -> p t n", t=2)
                            a_pair = at2[m][:, 256 * jp:256 * jp + 256].rearrange(
                                "p (t n) -> p t n", t=2)
                            nc.tensor.matmul(
                                out=pg[:, 128 * m:128 * m + 128],
                                lhsT=h_pair, rhs=a_pair,
                                start=(jp == 0), stop=(jp == 1), perf_mode=DR)
                            nc.tensor.matmul(
                                out=pgt[:, 128 * m:128 * m + 128],
                                lhsT=a_pair, rhs=h_pair,
                                start=(jp == 0), stop=(jp == 1), perf_mode=DR)

                    # maxes -> tanh -> exp weights
                    m_all = smp.tile([128, 5], F32, tag="ma")
                    nc.vector.reduce_max(out=m_all[:, 0:1], in_=pg[:], axis=AX)
                    nc.vector.reduce_max(
                        out=m_all[:, 1:5],
                        in_=pgt[:].rearrange("p (m q) -> p m q", m=4), axis=AX)
                    t_all = smp.tile([128, 5], F32, tag="ta")
                    nc.scalar.activation(out=t_all[:], in_=m_all[:],
                                         func=AF.Tanh, scale=SC_TANH)
                    e_all = smp.tile([128, 5], FP16, tag="ea")
                    nc.scalar.activation(out=e_all[:], in_=t_all[:], func=AF.Exp)

                    pending.append((b, yq_off, ya_offs, ygrp, e_all))
                    if len(pending) > 1:
                        emit_tail(*pending.pop(0))

            for p in pending:
                emit_tail(*p)

            # batched cosine-similarity finalize
            prod = cpool.tile([128, 256], BF16)
            nc.vector.tensor_mul(out=prod[:], in0=rball[:, 0:256],
                                 in1=rball[:, 256:512])
            sq = cpool.tile([128, 256], BF16)
            nc.vector.tensor_mul(out=sq[:], in0=rball[:, 0:256],
                                 in1=rball[:, 0:256])
            sa = cpool.tile([128, 256], BF16)
            nc.vector.tensor_mul(out=sa[:], in0=rball[:, 256:512],
                                 in1=rball[:, 256:512])
            dot = cpool.tile([1, 64], F32)
            q2r = cpool.tile([1, 64], F32)
            a2r = cpool.tile([1, 64], F32)
            for rhs_t, out_t in ((prod, dot), (sq, q2r), (sa, a2r)):
                pf = pfp.tile([1, 256], F32, tag="pf")
                nc.tensor.matmul(out=pf[:], lhsT=ones[:], rhs=rhs_t[:],
                                 start=True, stop=True)
                nc.vector.reduce_sum(
                    out=out_t[:],
                    in_=pf[:].rearrange("p (b c) -> p b c", c=4), axis=AX)
            den = cpool.tile([1, 64], F32)
            nc.vector.tensor_mul(out=den[:], in0=q2r[:], in1=a2r[:])
            sden = cpool.tile([1, 64], F32)
            nc.scalar.activation(out=sden[:], in_=den[:], func=AF.Sqrt)
            inv = cpool.tile([1, 64], F32)
            nc.vector.reciprocal(out=inv[:], in_=sden[:])
            res = cpool.tile([1, 64], F32)
            nc.vector.tensor_mul(out=res[:], in0=dot[:], in1=inv[:])
            nc.sync.dma_start(out_d.rearrange("(a b) -> a b", a=1), res[:])
    return nc


_BUILT = {}


def get_built():
    if "nc" not in _BUILT:
        nc = bacc.Bacc("TRN2", target_bir_lowering=False, debug=False,
                       num_devices=NCORES)
        build_kernel(nc)
        nc.compile()
        _BUILT["nc"] = nc
    return _BUILT["nc"]


def prep_inputs(question, answer, emb_table, conv_w, conv_b, U):
    f16 = np.float16
    f8 = ml_dtypes.float8_e4m3

    # host tables: C_k[v] = emb[v] @ w[:,:,k]^T, bias folded into C_1,
    # plus a zero sentinel row at index V1 for sequence edges.
    emb32 = emb_table.astype(np.float32)
    ccs = []
    for k in range(3):
        t = emb32 @ conv_w[:, :, k].astype(np.float32).T
        if k == 1:
            t = t + conv_b.astype(np.float32)[None, :]
        cc = np.zeros((VROWS, F), dtype=f16)
        cc[:V1] = t.astype(f16)
        ccs.append(cc)

    # U in fp8, x256, padded to [512, 512], packed so each DoubleRow pair of
    # f-chunks is a contiguous 256-col slice: cols [256*(2j+ip)] hold
    # [U8[f-chunk 2ip, g-chunk j] | U8[f-chunk 2ip+1, g-chunk j]].
    upad = np.zeros((512, 512), dtype=np.float32)
    upad[:F, :F] = U.astype(np.float32) * SC_U8
    u8 = upad.astype(f8)
    uu8 = np.zeros((128, 2048), dtype=f8)
    for j in range(4):
        for ip in range(2):
            for t in range(2):
                i = 2 * ip + t
                uu8[:, 256 * (2 * j + ip) + 128 * t:
                    256 * (2 * j + ip) + 128 * t + 128] = \
                    u8[128 * i:128 * i + 128, 128 * j:128 * j + 128]

    qi = question.astype(np.int64)
    ai = answer.astype(np.int64)
    in_maps = []
    for c in range(NCORES):
        qs = qi[c * BL:(c + 1) * BL]     # [BL, 128]
        as_ = ai[c * BL:(c + 1) * BL]    # [BL, 512]
        # token id at (elem, block, position p): block 0 = question,
        # blocks 1..4 = answer chunks of 128.
        tok = np.zeros((BL, NBLK, 128), dtype=np.int64)
        tok[:, 0, :] = qs
        tok[:, 1:, :] = as_.reshape(BL, 4, 128)
        # shifted index sections; sentinel = V1 (zero row)
        def shifted(seq, shift):  # seq [BL, L] -> [BL, L]
            out = np.full_like(seq, V1)
            if shift == -1:
                out[:, 1:] = seq[:, :-1]
            elif shift == 1:
                out[:, :-1] = seq[:, 1:]
            else:
                out = seq.copy()
            return out
        gidx = np.zeros((128, 3 * BL * NBLK), dtype=np.int32)
        for k in range(3):
            qsh = shifted(qs, k - 1)
            ash = shifted(as_, k - 1)
            toksh = np.zeros((BL, NBLK, 128), dtype=np.int64)
            toksh[:, 0, :] = qsh
            toksh[:, 1:, :] = ash.reshape(BL, 4, 128)
            # cols ordered (elem, block); partition = position in block
            sec = toksh.reshape(BL * NBLK, 128).T.astype(np.int32)
            gidx[:, k * BL * NBLK:(k + 1) * BL * NBLK] = sec
        in_maps.append({
            "cc0": ccs[0], "cc1": ccs[1], "cc2": ccs[2],
            "gidx": gidx, "uu8": uu8,
        })
    return in_maps


def kernel(question, answer, emb_table, conv_w, conv_b, U):
    question = np.asarray(question)
    answer = np.asarray(answer)
    emb_table = np.asarray(emb_table, dtype=np.float32)
    conv_w = np.asarray(conv_w, dtype=np.float32)
    conv_b = np.asarray(conv_b, dtype=np.float32)
    U = np.asarray(U, dtype=np.float32)

    nc = get_built()
    in_maps = prep_inputs(question, answer, emb_table, conv_w, conv_b, U)
    res = bass_utils.run_bass_kernel_spmd(nc, in_maps, core_ids=list(range(NCORES)))
    out = np.concatenate([np.asarray(res.results[c]["out"]).reshape(-1)
                          for c in range(NCORES)])
    return out.astype(np.float32)


# revision 6
# speedup vs baseline: 1.7735x; 1.0186x over previous
"""AttentivePoolingNetwork Trainium2 kernel, v2.

Data-parallel over batch across 8 NeuronCores (64 batch elements each).

Key idea: the conv1d is a per-token linear map of the embedding, so the
per-vocab contributions C_k[v] = W_k @ emb[v] (k = 3 taps, bias folded into
C_1) are precomputed host-side into fp16 tables. On device the conv output
y[l] = C_0[t(l-1)] + C_1[t(l)] + C_2[t(l+1)] is produced by three indirect
DMA gathers with CCE-add accumulation — no PE conv at all. A zero sentinel
row (index V) handles sequence edges.

Per batch element, on chip:
  y tiles [tokens, 400] fp16 (1 question block + 4 answer blocks)
  -> PE transposes + x4-scaled fp8 copies -> q8/at2 [f, tokens] fp8
  -> H = U^T Q, G = H^T A, G^T = A^T H as fp8 DoubleRow matmuls (2 k-tiles
     per instruction at 0.5 cycles/row)
  -> row maxes of G and G^T -> tanh -> exp -> pooling weights (softmax
     denominators cancel in the cosine similarity)
  -> pooled rQ/rA via N=1 matmuls (lhsT = y tiles, rhs = weight vectors)
  -> batched cosine-similarity finalize over all 64 elements.
"""

import numpy as np
import ml_dtypes

import concourse.bacc as bacc
import concourse.bass as bass
import concourse.tile as tile
import concourse.mybir as mybir
from concourse import bass_utils
from concourse.masks import make_identity

FP16 = mybir.dt.float16
FP8 = mybir.dt.float8e4
BF16 = mybir.dt.bfloat16
F32 = mybir.dt.float32
I32 = mybir.dt.int32
AX = mybir.AxisListType.X
AF = mybir.ActivationFunctionType
DR = mybir.MatmulPerfMode.DoubleRow
ADD = mybir.AluOpType.add
BYP = mybir.AluOpType.bypass

B, QL, AL = 512, 128, 512
V1, E, F = 50001, 300, 400
VROWS = V1 + 1          # + zero sentinel row for sequence edges
NCORES = 8
BL = B // NCORES        # 64 batch elements per core
M = 16                  # elements per gather group
NG = BL // M
NBLK = 5                # token blocks per element: 1 question + 4 answer

SC_Y8 = 4.0             # y -> fp8 scale on the G path
SC_H8 = 1.0 / 128.0     # H(dev) -> fp8 scale
SC_U8 = 256.0           # U -> fp8 scale (host)
SC_TANH = 1.0 / (SC_Y8 * SC_Y8 * SC_U8 * SC_H8)  # undo scales before tanh


def build_kernel(nc):
    cc0 = nc.dram_tensor("cc0", [VROWS, F], FP16, kind="ExternalInput").ap()
    cc1 = nc.dram_tensor("cc1", [VROWS, F], FP16, kind="ExternalInput").ap()
    cc2 = nc.dram_tensor("cc2", [VROWS, F], FP16, kind="ExternalInput").ap()
    gidx = nc.dram_tensor("gidx", [128, 3 * BL * NBLK], I32,
                          kind="ExternalInput").ap()
    uu8 = nc.dram_tensor("uu8", [128, 2048], FP8, kind="ExternalInput").ap()
    out_d = nc.dram_tensor("out", [BL], F32, kind="ExternalOutput").ap()
    ccs = (cc0, cc1, cc2)

    with tile.TileContext(nc) as tc:
        with (
            tc.tile_pool(name="const", bufs=1) as cpool,
            tc.tile_pool(name="yg", bufs=2) as ygp,
            tc.tile_pool(name="q8", bufs=2) as q8p,
            tc.tile_pool(name="at", bufs=8) as atp,
            tc.tile_pool(name="h8", bufs=2) as h8p,
            tc.tile_pool(name="sm", bufs=3) as smp,
            tc.tile_pool(name="ptr", bufs=2, space="PSUM") as ptr,
            tc.tile_pool(name="ph", bufs=2, space="PSUM") as php,
            tc.tile_pool(name="pg", bufs=1, space="PSUM") as pgp,
            tc.tile_pool(name="pt2", bufs=1, space="PSUM") as pt2p,
            tc.tile_pool(name="pr", bufs=1, space="PSUM") as prp,
            tc.tile_pool(name="pf", bufs=1, space="PSUM") as pfp,
        ):
            idn = cpool.tile([128, 128], FP16)
            make_identity(nc, idn[:])
            ix = cpool.tile([128, 3 * BL * NBLK], I32)
            nc.sync.dma_start(ix[:], gidx)
            uu = cpool.tile([128, 2048], FP8)
            nc.sync.dma_start(uu[:], uu8)
            ones = cpool.tile([128, 1], BF16)
            nc.vector.memset(ones[:], 1.0)
            # rq at cols [4b : 4b+4], ra at cols [256 + 4b : +4]; zero so the
            # partition tails of the 16-row f-chunk-3 columns stay zero.
            rball = cpool.tile([128, 512], F32)
            nc.vector.memset(rball[:], 0.0)

            YW = NBLK * F          # 2000 cols per element in a y group tile
            GW = M * YW            # 16000 cols per group

            def emit_tail(b, yq_off, ya_offs, ygrp, e_all):
                pr = prp.tile([128, 8], F32, tag="pr")
                for c in range(4):
                    w = 128 if c < 3 else 16
                    nc.tensor.matmul(
                        out=pr[0:w, c:c + 1],
                        lhsT=ygrp[:, yq_off + 128 * c: yq_off + 128 * c + w],
                        rhs=e_all[:, 0:1], start=True, stop=True)
                for c in range(4):
                    w = 128 if c < 3 else 16
                    for m in range(4):
                        nc.tensor.matmul(
                            out=pr[0:w, 4 + c:5 + c],
                            lhsT=ygrp[:, ya_offs[m] + 128 * c:
                                      ya_offs[m] + 128 * c + w],
                            rhs=e_all[:, 1 + m:2 + m],
                            start=(m == 0), stop=(m == 3))
                nc.gpsimd.tensor_copy(out=rball[:, 4 * b:4 * b + 3],
                                      in_=pr[:, 0:3])
                nc.gpsimd.tensor_copy(out=rball[0:16, 4 * b + 3:4 * b + 4],
                                      in_=pr[0:16, 3:4])
                nc.gpsimd.tensor_copy(out=rball[:, 256 + 4 * b:256 + 4 * b + 3],
                                      in_=pr[:, 4:7])
                nc.gpsimd.tensor_copy(out=rball[0:16, 256 + 4 * b + 3:
                                                256 + 4 * b + 4],
                                      in_=pr[0:16, 7:8])

            def issue_gathers(g):
                ygrp = ygp.tile([128, GW + 128], FP16, tag="yg")
                # keep the over-read tail of the chunk-3 transposes finite
                nc.vector.memset(ygrp[:, GW:GW + 128], 0.0)
                out_ap = ygrp[:, 0:GW].rearrange("p (n f) -> p n f", f=F)
                for k in (1, 0, 2):
                    sec = k * BL * NBLK + g * M * NBLK
                    nc.gpsimd.indirect_dma_start(
                        out=out_ap, out_offset=None, in_=ccs[k],
                        in_offset=bass.IndirectOffsetOnAxis(
                            ap=ix[:, sec:sec + M * NBLK], axis=0),
                        compute_op=(BYP if k == 1 else ADD))
                return ygrp

            pending = []
            ygs = {0: issue_gathers(0)}
            for g in range(NG):
                if g + 1 < NG:
                    ygs[g + 1] = issue_gathers(g + 1)
                ygrp = ygs.pop(g)

                for e in range(M):
                    b = g * M + e
                    yq_off = e * YW
                    ya_offs = [e * YW + (1 + m) * F for m in range(4)]

                    # transpose question block -> q8 [f, q] fp8 (x4)
                    tpq = ptr.tile([128, 512], FP16, tag="tp")
                    for c in range(4):
                        nc.tensor.transpose(
                            out=tpq[:, 128 * c:128 * c + 128],
                            in_=ygrp[:, yq_off + 128 * c:yq_off + 128 * c + 128],
                            identity=idn[:])
                    q8t = q8p.tile([128, 512], FP8, tag="q8")
                    nc.scalar.activation(out=q8t[:], in_=tpq[:], func=AF.Copy,
                                         scale=SC_Y8)

                    # transpose answer blocks -> at2[m] [g, a] fp8 (x4)
                    at2 = []
                    for m in range(4):
                        tpa = ptr.tile([128, 512], FP16, tag="tp")
                        for c in range(4):
                            nc.tensor.transpose(
                                out=tpa[:, 128 * c:128 * c + 128],
                                in_=ygrp[:, ya_offs[m] + 128 * c:
                                         ya_offs[m] + 128 * c + 128],
                                identity=idn[:])
                        at = atp.tile([128, 512], FP8, tag="at")
                        if m in (0, 1):
                            nc.vector.tensor_scalar_mul(out=at[:], in0=tpa[:],
                                                        scalar1=SC_Y8)
                        else:
                            nc.scalar.activation(out=at[:], in_=tpa[:],
                                                 func=AF.Copy, scale=SC_Y8)
                        at2.append(at)

                    # H[g, q] = sum_f U8[f, g] Q8[f, q]  (DoubleRow pairs)
                    ph = php.tile([128, 512], F32, tag="ph")
                    for j in range(4):
                        for ip in range(2):
                            nc.tensor.matmul(
                                out=ph[:, 128 * j:128 * j + 128],
                                lhsT=uu[:, 256 * (2 * j + ip):
                                        256 * (2 * j + ip) + 256].rearrange(
                                    "p (t m) -> p t m", t=2),
                                rhs=q8t[:, 256 * ip:256 * ip + 256].rearrange(
                                    "p (t n) -> p t n", t=2),
                                start=(ip == 0), stop=(ip == 1), perf_mode=DR)
                    h8t = h8p.tile([128, 512], FP8, tag="h8")
                    nc.scalar.activation(out=h8t[:], in_=ph[:], func=AF.Copy,
                                         scale=SC_H8)

                    # G[q, a] = sum_g H[g, q] A[g, a]; G^T likewise
                    pg = pgp.tile([128, 512], F32, tag="pg")
                    pgt = pt2p.tile([128, 512], F32, tag="pgt")
                    for m in range(4):
                        for jp in range(2):
                            h_pair = h8t[:, 256 * jp:256 * jp + 256].rearrange(
                                "p (t n) -> p t n", t=2)
                            a_pair = at2[m][:, 256 * jp:256 * jp + 256].rearrange(
                                "p (t n) -> p t n", t=2)
                            nc.tensor.matmul(
                                out=pg[:, 128 * m:128 * m + 128],
                                lhsT=h_pair, rhs=a_pair,
                                start=(jp == 0), stop=(jp == 1), perf_mode=DR)
                            nc.tensor.matmul(
                                out=pgt[:, 128 * m:128 * m + 128],
                                lhsT=a_pair, rhs=h_pair,
                                start=(jp == 0), stop=(jp == 1), perf_mode=DR)

                    # maxes -> tanh -> exp weights
                    m_all = smp.tile([128, 5], F32, tag="ma")
                    nc.vector.reduce_max(out=m_all[:, 0:1], in_=pg[:], axis=AX)
                    nc.vector.reduce_max(
                        out=m_all[:, 1:5],
                        in_=pgt[:].rearrange("p (m q) -> p m q", m=4), axis=AX)
                    t_all = smp.tile([128, 5], F32, tag="ta")
                    nc.scalar.activation(out=t_all[:], in_=m_all[:],
                                         func=AF.Tanh, scale=SC_TANH)
                    e_all = smp.tile([128, 5], FP16, tag="ea")
                    nc.scalar.activation(out=e_all[:], in_=t_all[:], func=AF.Exp)

                    pending.append((b, yq_off, ya_offs, ygrp, e_all))
                    if len(pending) > 1:
                        emit_tail(*pending.pop(0))

            for p in pending:
                emit_tail(*p)

            # batched cosine-similarity finalize
            prod = cpool.tile([128, 256], BF16)
            nc.vector.tensor_mul(out=prod[:], in0=rball[:, 0:256],
                                 in1=rball[:, 256:512])
            sq = cpool.tile([128, 256], BF16)
            nc.vector.tensor_mul(out=sq[:], in0=rball[:, 0:256],
                                 in1=rball[:, 0:256])
            sa = cpool.tile([128, 256], BF16)
            nc.vector.tensor_mul(out=sa[:], in0=rball[:, 256:512],
                                 in1=rball[:, 256:512])
            dot = cpool.tile([1, 64], F32)
            q2r = cpool.tile([1, 64], F32)
            a2r = cpool.tile([1, 64], F32)
            for rhs_t, out_t in ((prod, dot), (sq, q2r), (sa, a2r)):
                pf = pfp.tile([1, 256], F32, tag="pf")
                nc.tensor.matmul(out=pf[:], lhsT=ones[:], rhs=rhs_t[:],
                                 start=True, stop=True)
                nc.vector.reduce_sum(
                    out=out_t[:],
                    in_=pf[:].rearrange("p (b c) -> p b c", c=4), axis=AX)
            den = cpool.tile([1, 64], F32)
            nc.vector.tensor_mul(out=den[:], in0=q2r[:], in1=a2r[:])
            sden = cpool.tile([1, 64], F32)
            nc.scalar.activation(out=sden[:], in_=den[:], func=AF.Sqrt)
            inv = cpool.tile([1, 64], F32)
            nc.vector.reciprocal(out=inv[:], in_=sden[:])
            res = cpool.tile([1, 64], F32)
            nc.vector.tensor_mul(out=res[:], in0=dot[:], in1=inv[:])
            nc.sync.dma_start(out_d.rearrange("(a b) -> a b", a=1), res[:])
    return nc


_BUILT = {}


def get_built():
    if "nc" not in _BUILT:
        nc = bacc.Bacc("TRN2", target_bir_lowering=False, debug=False,
                       num_devices=NCORES)
        build_kernel(nc)
        nc.compile()
        _BUILT["nc"] = nc
    return _BUILT["nc"]


def prep_inputs(question, answer, emb_table, conv_w, conv_b, U):
    f16 = np.float16
    f8 = ml_dtypes.float8_e4m3

    # host tables: C_k[v] = emb[v] @ w[:,:,k]^T, bias folded into C_1,
    # plus a zero sentinel row at index V1 for sequence edges.
    emb32 = emb_table.astype(np.float32)
    ccs = []
    for k in range(3):
        t = emb32 @ conv_w[:, :, k].astype(np.float32).T
        if k == 1:
            t = t + conv_b.astype(np.float32)[None, :]
        cc = np.zeros((VROWS, F), dtype=f16)
        cc[:V1] = t.astype(f16)
        ccs.append(cc)

    # U in fp8, x256, padded to [512, 512], packed so each DoubleRow pair of
    # f-chunks is a contiguous 256-col slice: cols [256*(2j+ip)] hold
    # [U8[f-chunk 2ip, g-chunk j] | U8[f-chunk 2ip+1, g-chunk j]].
    upad = np.zeros((512, 512), dtype=np.float32)
    upad[:F, :F] = U.astype(np.float32) * SC_U8
    u8 = upad.astype(f8)
    uu8 = np.zeros((128, 2048), dtype=f8)
    for j in range(4):
        for ip in range(2):
            for t in range(2):
                i = 2 * ip + t
                uu8[:, 256 * (2 * j + ip) + 128 * t:
                    256 * (2 * j + ip) + 128 * t + 128] = \
                    u8[128 * i:128 * i + 128, 128 * j:128 * j + 128]

    qi = question.astype(np.int64)
    ai = answer.astype(np.int64)
    in_maps = []
    for c in range(NCORES):
        qs = qi[c * BL:(c + 1) * BL]     # [BL, 128]
        as_ = ai[c * BL:(c + 1) * BL]    # [BL, 512]
        # token id at (elem, block, position p): block 0 = question,
        # blocks 1..4 = answer chunks of 128.
        tok = np.zeros((BL, NBLK, 128), dtype=np.int64)
        tok[:, 0, :] = qs
        tok[:, 1:, :] = as_.reshape(BL, 4, 128)
        # shifted index sections; sentinel = V1 (zero row)
        def shifted(seq, shift):  # seq [BL, L] -> [BL, L]
            out = np.full_like(seq, V1)
            if shift == -1:
                out[:, 1:] = seq[:, :-1]
            elif shift == 1:
                out[:, :-1] = seq[:, 1:]
            else:
                out = seq.copy()
            return out
        gidx = np.zeros((128, 3 * BL * NBLK), dtype=np.int32)
        for k in range(3):
            qsh = shifted(qs, k - 1)
            ash = shifted(as_, k - 1)
            toksh = np.zeros((BL, NBLK, 128), dtype=np.int64)
            toksh[:, 0, :] = qsh
            toksh[:, 1:, :] = ash.reshape(BL, 4, 128)
            # cols ordered (elem, block); partition = position in block
            sec = toksh.reshape(BL * NBLK, 128).T.astype(np.int32)
            gidx[:, k * BL * NBLK:(k + 1) * BL * NBLK] = sec
        in_maps.append({
            "cc0": ccs[0], "cc1": ccs[1], "cc2": ccs[2],
            "gidx": gidx, "uu8": uu8,
        })
    return in_maps


def kernel(question, answer, emb_table, conv_w, conv_b, U):
    question = np.asarray(question)
    answer = np.asarray(answer)
    emb_table = np.asarray(emb_table, dtype=np.float32)
    conv_w = np.asarray(conv_w, dtype=np.float32)
    conv_b = np.asarray(conv_b, dtype=np.float32)
    U = np.asarray(U, dtype=np.float32)

    nc = get_built()
    in_maps = prep_inputs(question, answer, emb_table, conv_w, conv_b, U)
    res = bass_utils.run_bass_kernel_spmd(nc, in_maps, core_ids=list(range(NCORES)))
    out = np.concatenate([np.asarray(res.results[c]["out"]).reshape(-1)
                          for c in range(NCORES)])
    return out.astype(np.float32)


# revision 7
# speedup vs baseline: 1.8437x; 1.0396x over previous
"""AttentivePoolingNetwork Trainium2 kernel, v2.

Data-parallel over batch across 8 NeuronCores (64 batch elements each).

Key idea: the conv1d is a per-token linear map of the embedding, so the
per-vocab contributions C_k[v] = W_k @ emb[v] (k = 3 taps, bias folded into
C_1) are precomputed host-side into fp16 tables. On device the conv output
y[l] = C_0[t(l-1)] + C_1[t(l)] + C_2[t(l+1)] is produced by three indirect
DMA gathers with CCE-add accumulation — no PE conv at all. A zero sentinel
row (index V) handles sequence edges.

Per batch element, on chip:
  y tiles [tokens, 400] fp16 (1 question block + 4 answer blocks)
  -> PE transposes + x4-scaled fp8 copies -> q8/at2 [f, tokens] fp8
  -> H = U^T Q, G = H^T A, G^T = A^T H as fp8 DoubleRow matmuls (2 k-tiles
     per instruction at 0.5 cycles/row)
  -> row maxes of G and G^T -> tanh -> exp -> pooling weights (softmax
     denominators cancel in the cosine similarity)
  -> pooled rQ/rA via N=1 matmuls (lhsT = y tiles, rhs = weight vectors)
  -> batched cosine-similarity finalize over all 64 elements.
"""

import numpy as np
import ml_dtypes

import concourse.bacc as bacc
import concourse.bass as bass
import concourse.tile as tile
import concourse.mybir as mybir
from concourse import bass_utils
from concourse.masks import make_identity

FP16 = mybir.dt.float16
FP8 = mybir.dt.float8e4
BF16 = mybir.dt.bfloat16
F32 = mybir.dt.float32
I32 = mybir.dt.int32
AX = mybir.AxisListType.X
AF = mybir.ActivationFunctionType
DR = mybir.MatmulPerfMode.DoubleRow
ADD = mybir.AluOpType.add
BYP = mybir.AluOpType.bypass

B, QL, AL = 512, 128, 512
V1, E, F = 50001, 300, 400
VROWS = V1 + 1          # + zero sentinel row for sequence edges
NCORES = 8
BL = B // NCORES        # 64 batch elements per core
M = 16                  # elements per gather group
NG = BL // M
NBLK = 5                # token blocks per element: 1 question + 4 answer

SC_Y8 = 4.0             # y -> fp8 scale on the G path
SC_H8 = 1.0 / 128.0     # H(dev) -> fp8 scale
SC_U8 = 256.0           # U -> fp8 scale (host)
SC_TANH = 1.0 / (SC_Y8 * SC_Y8 * SC_U8 * SC_H8)  # undo scales before tanh


def build_kernel(nc):
    cc0 = nc.dram_tensor("cc0", [VROWS, F], FP16, kind="ExternalInput").ap()
    cc1 = nc.dram_tensor("cc1", [VROWS, F], FP16, kind="ExternalInput").ap()
    cc2 = nc.dram_tensor("cc2", [VROWS, F], FP16, kind="ExternalInput").ap()
    gidx = nc.dram_tensor("gidx", [128, 3 * BL * NBLK], I32,
                          kind="ExternalInput").ap()
    uu8 = nc.dram_tensor("uu8", [128, 2048], FP8, kind="ExternalInput").ap()
    out_d = nc.dram_tensor("out", [BL], F32, kind="ExternalOutput").ap()
    ccs = (cc0, cc1, cc2)

    with tile.TileContext(nc) as tc:
        with (
            tc.tile_pool(name="const", bufs=1) as cpool,
            tc.tile_pool(name="yg", bufs=2) as ygp,
            tc.tile_pool(name="q8", bufs=2) as q8p,
            tc.tile_pool(name="at", bufs=8) as atp,
            tc.tile_pool(name="h8", bufs=2) as h8p,
            tc.tile_pool(name="sm", bufs=3) as smp,
            tc.tile_pool(name="ptr", bufs=2, space="PSUM") as ptr,
            tc.tile_pool(name="ph", bufs=2, space="PSUM") as php,
            tc.tile_pool(name="pg", bufs=1, space="PSUM") as pgp,
            tc.tile_pool(name="pt2", bufs=1, space="PSUM") as pt2p,
            tc.tile_pool(name="pr", bufs=1, space="PSUM") as prp,
            tc.tile_pool(name="pf", bufs=1, space="PSUM") as pfp,
        ):
            idn = cpool.tile([128, 128], FP16)
            make_identity(nc, idn[:])
            ix = cpool.tile([128, 3 * BL * NBLK], I32)
            nc.sync.dma_start(ix[:], gidx)
            uu = cpool.tile([128, 2048], FP8)
            nc.sync.dma_start(uu[:], uu8)
            ones = cpool.tile([128, 1], BF16)
            nc.vector.memset(ones[:], 1.0)
            # interleaved: elem b holds rq chunks at cols [8b..8b+4) and
            # ra chunks at [8b+4..8b+8)
            rball = cpool.tile([128, 512], F32)

            YW = NBLK * F          # 2000 cols per element in a y group tile
            GW = M * YW            # 16000 cols per group

            def emit_tail(b, yq_off, ya_offs, ygrp, e_all):
                # chunk 3 uses the full 128-wide lhsT (over-reads into the
                # next block / zeroed tail); the garbage rows 16:128 of its
                # output column are excluded by the masked finalize.
                pr = prp.tile([128, 8], F32, tag="pr")
                for c in range(4):
                    nc.tensor.matmul(
                        out=pr[:, c:c + 1],
                        lhsT=ygrp[:, yq_off + 128 * c: yq_off + 128 * c + 128],
                        rhs=e_all[:, 0:1], start=True, stop=True)
                for c in range(4):
                    for m in range(4):
                        nc.tensor.matmul(
                            out=pr[:, 4 + c:5 + c],
                            lhsT=ygrp[:, ya_offs[m] + 128 * c:
                                      ya_offs[m] + 128 * c + 128],
                            rhs=e_all[:, 1 + m:2 + m],
                            start=(m == 0), stop=(m == 3))
                nc.vector.tensor_copy(out=rball[:, 8 * b:8 * b + 8],
                                      in_=pr[:])

            def issue_gathers(g):
                ygrp = ygp.tile([128, GW + 128], FP16, tag="yg")
                # keep the over-read tail of the chunk-3 transposes finite
                nc.vector.memset(ygrp[:, GW:GW + 128], 0.0)
                out_ap = ygrp[:, 0:GW].rearrange("p (n f) -> p n f", f=F)
                for k in (1, 0, 2):
                    sec = k * BL * NBLK + g * M * NBLK
                    nc.gpsimd.indirect_dma_start(
                        out=out_ap, out_offset=None, in_=ccs[k],
                        in_offset=bass.IndirectOffsetOnAxis(
                            ap=ix[:, sec:sec + M * NBLK], axis=0),
                        compute_op=(BYP if k == 1 else ADD))
                return ygrp

            pending = []
            ygs = {0: issue_gathers(0)}
            for g in range(NG):
                if g + 1 < NG:
                    ygs[g + 1] = issue_gathers(g + 1)
                ygrp = ygs.pop(g)

                for e in range(M):
                    b = g * M + e
                    yq_off = e * YW
                    ya_offs = [e * YW + (1 + m) * F for m in range(4)]

                    # transpose question block -> q8 [f, q] fp8 (x4)
                    tpq = ptr.tile([128, 512], FP16, tag="tp")
                    for c in range(4):
                        nc.tensor.transpose(
                            out=tpq[:, 128 * c:128 * c + 128],
                            in_=ygrp[:, yq_off + 128 * c:yq_off + 128 * c + 128],
                            identity=idn[:])
                    q8t = q8p.tile([128, 512], FP8, tag="q8")
                    nc.scalar.activation(out=q8t[:], in_=tpq[:], func=AF.Copy,
                                         scale=SC_Y8)

                    # transpose answer blocks -> at2[m] [g, a] fp8 (x4)
                    at2 = []
                    for m in range(4):
                        tpa = ptr.tile([128, 512], FP16, tag="tp")
                        for c in range(4):
                            nc.tensor.transpose(
                                out=tpa[:, 128 * c:128 * c + 128],
                                in_=ygrp[:, ya_offs[m] + 128 * c:
                                         ya_offs[m] + 128 * c + 128],
                                identity=idn[:])
                        at = atp.tile([128, 512], FP8, tag="at")
                        if m in (0, 1):
                            nc.vector.tensor_scalar_mul(out=at[:], in0=tpa[:],
                                                        scalar1=SC_Y8)
                        else:
                            nc.scalar.activation(out=at[:], in_=tpa[:],
                                                 func=AF.Copy, scale=SC_Y8)
                        at2.append(at)

                    # H[g, q] = sum_f U8[f, g] Q8[f, q]  (DoubleRow pairs)
                    ph = php.tile([128, 512], F32, tag="ph")
                    for j in range(4):
                        for ip in range(2):
                            nc.tensor.matmul(
                                out=ph[:, 128 * j:128 * j + 128],
                                lhsT=uu[:, 256 * (2 * j + ip):
                                        256 * (2 * j + ip) + 256].rearrange(
                                    "p (t m) -> p t m", t=2),
                                rhs=q8t[:, 256 * ip:256 * ip + 256].rearrange(
                                    "p (t n) -> p t n", t=2),
                                start=(ip == 0), stop=(ip == 1), perf_mode=DR)
                    h8t = h8p.tile([128, 512], FP8, tag="h8")
                    nc.scalar.activation(out=h8t[:], in_=ph[:], func=AF.Copy,
                                         scale=SC_H8)

                    # G[q, a] = sum_g H[g, q] A[g, a]; G^T likewise
                    pg = pgp.tile([128, 512], F32, tag="pg")
                    pgt = pt2p.tile([128, 512], F32, tag="pgt")
                    for m in range(4):
                        for jp in range(2):
                            h_pair = h8t[:, 256 * jp:256 * jp + 256].rearrange(
                                "p (t n) -> p t n", t=2)
                            a_pair = at2[m][:, 256 * jp:256 * jp + 256].rearrange(
                                "p (t n) -> p t n", t=2)
                            nc.tensor.matmul(
                                out=pg[:, 128 * m:128 * m + 128],
                                lhsT=h_pair, rhs=a_pair,
                                start=(jp == 0), stop=(jp == 1), perf_mode=DR)
                            nc.tensor.matmul(
                                out=pgt[:, 128 * m:128 * m + 128],
                                lhsT=a_pair, rhs=h_pair,
                                start=(jp == 0), stop=(jp == 1), perf_mode=DR)

                    # maxes -> tanh -> exp weights
                    m_all = smp.tile([128, 5], F32, tag="ma")
                    nc.vector.reduce_max(out=m_all[:, 0:1], in_=pg[:], axis=AX)
                    nc.vector.reduce_max(
                        out=m_all[:, 1:5],
                        in_=pgt[:].rearrange("p (m q) -> p m q", m=4), axis=AX)
                    t_all = smp.tile([128, 5], F32, tag="ta")
                    nc.scalar.activation(out=t_all[:], in_=m_all[:],
                                         func=AF.Tanh, scale=SC_TANH)
                    e_all = smp.tile([128, 5], FP16, tag="ea")
                    nc.scalar.activation(out=e_all[:], in_=t_all[:], func=AF.Exp)

                    pending.append((b, yq_off, ya_offs, ygrp, e_all))
                    if len(pending) > 1:
                        emit_tail(*pending.pop(0))

            for p in pending:
                emit_tail(*p)

            # batched cosine-similarity finalize (masked: f-chunk-3 columns
            # only use partitions 0:16)
            rb3 = rball[:].rearrange("p (b k) -> p b k", k=8)
            rq_ap = rb3[:, :, 0:4]
            ra_ap = rb3[:, :, 4:8]
            prod = cpool.tile([128, 256], BF16)
            sq = cpool.tile([128, 256], BF16)
            sa = cpool.tile([128, 256], BF16)
            nc.vector.tensor_mul(out=prod[:], in0=rq_ap, in1=ra_ap)
            nc.vector.tensor_mul(out=sq[:], in0=rq_ap, in1=rq_ap)
            nc.vector.tensor_mul(out=sa[:], in0=ra_ap, in1=ra_ap)
            dot = cpool.tile([1, 64], F32)
            q2r = cpool.tile([1, 64], F32)
            a2r = cpool.tile([1, 64], F32)
            for src_t, out_t in ((prod, dot), (sq, q2r), (sa, a2r)):
                s3 = src_t[:].rearrange("p (b c) -> p b c", c=4)
                pf = pfp.tile([1, 256], F32, tag="pf")
                # chunks 0..2 over all 128 partitions
                nc.tensor.matmul(out=pf[:, 0:192], lhsT=ones[:],
                                 rhs=s3[:, :, 0:3], start=True, stop=True)
                # chunk 3 over partitions 0:16 only
                nc.tensor.matmul(out=pf[:, 192:256], lhsT=ones[0:16, :],
                                 rhs=s3[0:16, :, 3:4], start=True, stop=True)
                part = cpool.tile([1, 64], F32, tag="fin_part")
                nc.vector.reduce_sum(
                    out=part[:],
                    in_=pf[:, 0:192].rearrange("p (b c) -> p b c", c=3),
                    axis=AX)
                nc.vector.tensor_add(out=out_t[:], in0=part[:],
                                     in1=pf[:, 192:256])
            den = cpool.tile([1, 64], F32)
            nc.vector.tensor_mul(out=den[:], in0=q2r[:], in1=a2r[:])
            sden = cpool.tile([1, 64], F32)
            nc.scalar.activation(out=sden[:], in_=den[:], func=AF.Sqrt)
            inv = cpool.tile([1, 64], F32)
            nc.vector.reciprocal(out=inv[:], in_=sden[:])
            res = cpool.tile([1, 64], F32)
            nc.vector.tensor_mul(out=res[:], in0=dot[:], in1=inv[:])
            nc.sync.dma_start(out_d.rearrange("(a b) -> a b", a=1), res[:])
    return nc


_BUILT = {}


def get_built():
    if "nc" not in _BUILT:
        nc = bacc.Bacc("TRN2", target_bir_lowering=False, debug=False,
                       num_devices=NCORES)
        build_kernel(nc)
        nc.compile()
        _BUILT["nc"] = nc
    return _BUILT["nc"]


def prep_inputs(question, answer, emb_table, conv_w, conv_b, U):
    f16 = np.float16
    f8 = ml_dtypes.float8_e4m3

    # host tables: C_k[v] = emb[v] @ w[:,:,k]^T, bias folded into C_1,
    # plus a zero sentinel row at index V1 for sequence edges.
    emb32 = emb_table.astype(np.float32)
    ccs = []
    for k in range(3):
        t = emb32 @ conv_w[:, :, k].astype(np.float32).T
        if k == 1:
            t = t + conv_b.astype(np.float32)[None, :]
        cc = np.zeros((VROWS, F), dtype=f16)
        cc[:V1] = t.astype(f16)
        ccs.append(cc)

    # U in fp8, x256, padded to [512, 512], packed so each DoubleRow pair of
    # f-chunks is a contiguous 256-col slice: cols [256*(2j+ip)] hold
    # [U8[f-chunk 2ip, g-chunk j] | U8[f-chunk 2ip+1, g-chunk j]].
    upad = np.zeros((512, 512), dtype=np.float32)
    upad[:F, :F] = U.astype(np.float32) * SC_U8
    u8 = upad.astype(f8)
    uu8 = np.zeros((128, 2048), dtype=f8)
    for j in range(4):
        for ip in range(2):
            for t in range(2):
                i = 2 * ip + t
                uu8[:, 256 * (2 * j + ip) + 128 * t:
                    256 * (2 * j + ip) + 128 * t + 128] = \
                    u8[128 * i:128 * i + 128, 128 * j:128 * j + 128]

    qi = question.astype(np.int64)
    ai = answer.astype(np.int64)
    in_maps = []
    for c in range(NCORES):
        qs = qi[c * BL:(c + 1) * BL]     # [BL, 128]
        as_ = ai[c * BL:(c + 1) * BL]    # [BL, 512]
        # token id at (elem, block, position p): block 0 = question,
        # blocks 1..4 = answer chunks of 128.
        tok = np.zeros((BL, NBLK, 128), dtype=np.int64)
        tok[:, 0, :] = qs
        tok[:, 1:, :] = as_.reshape(BL, 4, 128)
        # shifted index sections; sentinel = V1 (zero row)
        def shifted(seq, shift):  # seq [BL, L] -> [BL, L]
            out = np.full_like(seq, V1)
            if shift == -1:
                out[:, 1:] = seq[:, :-1]
            elif shift == 1:
                out[:, :-1] = seq[:, 1:]
            else:
                out = seq.copy()
            return out
        gidx = np.zeros((128, 3 * BL * NBLK), dtype=np.int32)
        for k in range(3):
            qsh = shifted(qs, k - 1)
            ash = shifted(as_, k - 1)
            toksh = np.zeros((BL, NBLK, 128), dtype=np.int64)
            toksh[:, 0, :] = qsh
            toksh[:, 1:, :] = ash.reshape(BL, 4, 128)
            # cols ordered (elem, block); partition = position in block
            sec = toksh.reshape(BL * NBLK, 128).T.astype(np.int32)
            gidx[:, k * BL * NBLK:(k + 1) * BL * NBLK] = sec
        in_maps.append({
            "cc0": ccs[0], "cc1": ccs[1], "cc2": ccs[2],
            "gidx": gidx, "uu8": uu8,
        })
    return in_maps


def kernel(question, answer, emb_table, conv_w, conv_b, U):
    question = np.asarray(question)
    answer = np.asarray(answer)
    emb_table = np.asarray(emb_table, dtype=np.float32)
    conv_w = np.asarray(conv_w, dtype=np.float32)
    conv_b = np.asarray(conv_b, dtype=np.float32)
    U = np.asarray(U, dtype=np.float32)

    nc = get_built()
    in_maps = prep_inputs(question, answer, emb_table, conv_w, conv_b, U)
    res = bass_utils.run_bass_kernel_spmd(nc, in_maps, core_ids=list(range(NCORES)))
    out = np.concatenate([np.asarray(res.results[c]["out"]).reshape(-1)
                          for c in range(NCORES)])
    return out.astype(np.float32)


# revision 9
# speedup vs baseline: 7.4366x; 4.0334x over previous
"""AttentivePoolingNetwork Trainium2 kernel, v2.

Data-parallel over batch across 8 NeuronCores (64 batch elements each).

Key idea: the conv1d is a per-token linear map of the embedding, so the
per-vocab contributions C_k[v] = W_k @ emb[v] (k = 3 taps, bias folded into
C_1) are precomputed host-side into fp16 tables. On device the conv output
y[l] = C_0[t(l-1)] + C_1[t(l)] + C_2[t(l+1)] is produced by three indirect
DMA gathers with CCE-add accumulation — no PE conv at all. A zero sentinel
row (index V) handles sequence edges.

Per batch element, on chip:
  y tiles [tokens, 400] fp16 (1 question block + 4 answer blocks)
  -> PE transposes + x4-scaled fp8 copies -> q8/at2 [f, tokens] fp8
  -> H = U^T Q, G = H^T A, G^T = A^T H as fp8 DoubleRow matmuls (2 k-tiles
     per instruction at 0.5 cycles/row)
  -> row maxes of G and G^T -> tanh -> exp -> pooling weights (softmax
     denominators cancel in the cosine similarity)
  -> pooled rQ/rA via N=1 matmuls (lhsT = y tiles, rhs = weight vectors)
  -> batched cosine-similarity finalize over all 64 elements.
"""

import numpy as np
import ml_dtypes

import concourse.bacc as bacc
import concourse.bass as bass
import concourse.tile as tile
import concourse.mybir as mybir
from concourse import bass_utils
from concourse.masks import make_identity

FP16 = mybir.dt.float16
FP8 = mybir.dt.float8e4
BF16 = mybir.dt.bfloat16
F32 = mybir.dt.float32
I32 = mybir.dt.int32
AX = mybir.AxisListType.X
AF = mybir.ActivationFunctionType
DR = mybir.MatmulPerfMode.DoubleRow
ADD = mybir.AluOpType.add
BYP = mybir.AluOpType.bypass

B, QL, AL = 512, 128, 512
V1, E, F = 50001, 300, 400
VROWS = V1 + 1          # + zero sentinel row for sequence edges
NCORES = 8
BL = B // NCORES        # 64 batch elements per core
import os
M = int(os.environ.get("KM", 8))  # elements per gather group
NG = BL // M
NBLK = 5                # token blocks per element: 1 question + 4 answer

SC_Y8 = 4.0             # y -> fp8 scale on the G path
SC_H8 = 1.0 / 128.0     # H(dev) -> fp8 scale
SC_U8 = 256.0           # U -> fp8 scale (host)
SC_TANH = 1.0 / (SC_Y8 * SC_Y8 * SC_U8 * SC_H8)  # undo scales before tanh


def build_kernel(nc):
    cc0 = nc.dram_tensor("cc0", [VROWS, F], FP16, kind="ExternalInput").ap()
    cc1 = nc.dram_tensor("cc1", [VROWS, F], FP16, kind="ExternalInput").ap()
    cc2 = nc.dram_tensor("cc2", [VROWS, F], FP16, kind="ExternalInput").ap()
    gidx = nc.dram_tensor("gidx", [128, 3 * BL * NBLK], I32,
                          kind="ExternalInput").ap()
    uu8 = nc.dram_tensor("uu8", [128, 2048], FP8, kind="ExternalInput").ap()
    out_d = nc.dram_tensor("out", [BL], F32, kind="ExternalOutput").ap()
    ccs = (cc0, cc1, cc2)

    with tile.TileContext(nc) as tc:
        with (
            tc.tile_pool(name="const", bufs=1) as cpool,
            tc.tile_pool(name="yg", bufs=2) as ygp,
            tc.tile_pool(name="q8", bufs=2) as q8p,
            tc.tile_pool(name="at", bufs=8) as atp,
            tc.tile_pool(name="h8", bufs=2) as h8p,
            tc.tile_pool(name="sm", bufs=3) as smp,
            tc.tile_pool(name="ptr", bufs=2, space="PSUM") as ptr,
            tc.tile_pool(name="ph", bufs=2, space="PSUM") as php,
            tc.tile_pool(name="pg", bufs=2, space="PSUM") as pgp,
            tc.tile_pool(name="pt2", bufs=2, space="PSUM") as pt2p,
        ):
            idn = cpool.tile([128, 128], FP16)
            make_identity(nc, idn[:])
            ix = cpool.tile([128, 3 * BL * NBLK], I32)
            nc.sync.dma_start(ix[:], gidx)
            uu = cpool.tile([128, 2048], FP8)
            nc.sync.dma_start(uu[:], uu8)
            ones = cpool.tile([128, 1], BF16)
            nc.vector.memset(ones[:], 1.0)
            # interleaved: elem b holds rq chunks at cols [8b..8b+4) and
            # ra chunks at [8b+4..8b+8)
            rball = cpool.tile([128, 512], F32)

            YW = NBLK * F          # 2000 cols per element in a y group tile
            GW = M * YW            # 16000 cols per group

            def emit_tail(b, yq_off, ya_offs, ygrp, e_all):
                # chunk 3 uses the full 128-wide lhsT (over-reads into the
                # next block / zeroed tail); the garbage rows 16:128 of its
                # output column are excluded by the masked finalize.
                pr = php.tile([128, 8], F32, tag="ph")
                for c in range(4):
                    nc.tensor.matmul(
                        out=pr[:, c:c + 1],
                        lhsT=ygrp[:, yq_off + 128 * c: yq_off + 128 * c + 128],
                        rhs=e_all[:, 0:1], start=True, stop=True)
                for c in range(4):
                    for m in range(4):
                        nc.tensor.matmul(
                            out=pr[:, 4 + c:5 + c],
                            lhsT=ygrp[:, ya_offs[m] + 128 * c:
                                      ya_offs[m] + 128 * c + 128],
                            rhs=e_all[:, 1 + m:2 + m],
                            start=(m == 0), stop=(m == 3))
                nc.vector.tensor_copy(out=rball[:, 8 * b:8 * b + 8],
                                      in_=pr[:])

            def issue_gathers(g):
                ygrp = ygp.tile([128, GW + 128], FP16, tag="yg")
                # keep the over-read tail of the chunk-3 transposes finite
                nc.vector.memset(ygrp[:, GW:GW + 128], 0.0)
                out_ap = ygrp[:, 0:GW].rearrange("p (n f) -> p n f", f=F)
                for k in (1, 0, 2):
                    sec = k * BL * NBLK + g * M * NBLK
                    nc.gpsimd.indirect_dma_start(
                        out=out_ap, out_offset=None, in_=ccs[k],
                        in_offset=bass.IndirectOffsetOnAxis(
                            ap=ix[:, sec:sec + M * NBLK], axis=0),
                        compute_op=(BYP if k == 1 else ADD))
                return ygrp

            pending = []
            ygs = {0: issue_gathers(0)}
            for g in range(NG):
                if g + 1 < NG:
                    ygs[g + 1] = issue_gathers(g + 1)
                ygrp = ygs.pop(g)

                for e in range(M):
                    b = g * M + e
                    yq_off = e * YW
                    ya_offs = [e * YW + (1 + m) * F for m in range(4)]

                    # transpose question block -> q8 [f, q] fp8 (x4)
                    tpq = ptr.tile([128, 512], FP16, tag="tp")
                    for c in range(4):
                        nc.tensor.transpose(
                            out=tpq[:, 128 * c:128 * c + 128],
                            in_=ygrp[:, yq_off + 128 * c:yq_off + 128 * c + 128],
                            identity=idn[:])
                    q8t = q8p.tile([128, 512], FP8, tag="q8")
                    nc.scalar.activation(out=q8t[:], in_=tpq[:], func=AF.Copy,
                                         scale=SC_Y8)

                    # transpose answer blocks -> at2[m] [g, a] fp8 (x4)
                    at2 = []
                    for m in range(4):
                        tpa = ptr.tile([128, 512], FP16, tag="tp")
                        for c in range(4):
                            nc.tensor.transpose(
                                out=tpa[:, 128 * c:128 * c + 128],
                                in_=ygrp[:, ya_offs[m] + 128 * c:
                                         ya_offs[m] + 128 * c + 128],
                                identity=idn[:])
                        at = atp.tile([128, 512], FP8, tag="at")
                        if m in (0, 1):
                            nc.vector.tensor_scalar_mul(out=at[:], in0=tpa[:],
                                                        scalar1=SC_Y8)
                        else:
                            nc.scalar.activation(out=at[:], in_=tpa[:],
                                                 func=AF.Copy, scale=SC_Y8)
                        at2.append(at)

                    # H[g, q] = sum_f U8[f, g] Q8[f, q]  (DoubleRow pairs)
                    ph = php.tile([128, 512], F32, tag="ph")
                    for j in range(4):
                        for ip in range(2):
                            nc.tensor.matmul(
                                out=ph[:, 128 * j:128 * j + 128],
                                lhsT=uu[:, 256 * (2 * j + ip):
                                        256 * (2 * j + ip) + 256].rearrange(
                                    "p (t m) -> p t m", t=2),
                                rhs=q8t[:, 256 * ip:256 * ip + 256].rearrange(
                                    "p (t n) -> p t n", t=2),
                                start=(ip == 0), stop=(ip == 1), perf_mode=DR)
                    h8t = h8p.tile([128, 512], FP8, tag="h8")
                    nc.scalar.activation(out=h8t[:], in_=ph[:], func=AF.Copy,
                                         scale=SC_H8)

                    # G[q, a] = sum_g H[g, q] A[g, a]; G^T likewise
                    pg = pgp.tile([128, 512], F32, tag="pg")
                    pgt = pt2p.tile([128, 512], F32, tag="pgt")
                    for m in range(4):
                        for jp in range(2):
                            h_pair = h8t[:, 256 * jp:256 * jp + 256].rearrange(
                                "p (t n) -> p t n", t=2)
                            a_pair = at2[m][:, 256 * jp:256 * jp + 256].rearrange(
                                "p (t n) -> p t n", t=2)
                            nc.tensor.matmul(
                                out=pg[:, 128 * m:128 * m + 128],
                                lhsT=h_pair, rhs=a_pair,
                                start=(jp == 0), stop=(jp == 1), perf_mode=DR)
                            nc.tensor.matmul(
                                out=pgt[:, 128 * m:128 * m + 128],
                                lhsT=a_pair, rhs=h_pair,
                                start=(jp == 0), stop=(jp == 1), perf_mode=DR)

                    # maxes -> tanh -> exp weights
                    m_all = smp.tile([128, 5], F32, tag="ma")
                    nc.vector.reduce_max(out=m_all[:, 0:1], in_=pg[:], axis=AX)
                    nc.vector.reduce_max(
                        out=m_all[:, 1:5],
                        in_=pgt[:].rearrange("p (m q) -> p m q", m=4), axis=AX)
                    t_all = smp.tile([128, 5], F32, tag="ta")
                    nc.scalar.activation(out=t_all[:], in_=m_all[:],
                                         func=AF.Tanh, scale=SC_TANH)
                    e_all = smp.tile([128, 5], FP16, tag="ea")
                    nc.scalar.activation(out=e_all[:], in_=t_all[:], func=AF.Exp)

                    pending.append((b, yq_off, ya_offs, ygrp, e_all))
                    if len(pending) > 1:
                        emit_tail(*pending.pop(0))

            for p in pending:
                emit_tail(*p)

            # batched cosine-similarity finalize (masked: f-chunk-3 columns
            # only use partitions 0:16)
            rb3 = rball[:].rearrange("p (b k) -> p b k", k=8)
            rq_ap = rb3[:, :, 0:4]
            ra_ap = rb3[:, :, 4:8]
            prod = cpool.tile([128, 256], BF16)
            sq = cpool.tile([128, 256], BF16)
            sa = cpool.tile([128, 256], BF16)
            nc.vector.tensor_mul(out=prod[:], in0=rq_ap, in1=ra_ap)
            nc.vector.tensor_mul(out=sq[:], in0=rq_ap, in1=rq_ap)
            nc.vector.tensor_mul(out=sa[:], in0=ra_ap, in1=ra_ap)
            dot = cpool.tile([1, 64], F32)
            q2r = cpool.tile([1, 64], F32)
            a2r = cpool.tile([1, 64], F32)
            for src_t, out_t in ((prod, dot), (sq, q2r), (sa, a2r)):
                s3 = src_t[:].rearrange("p (b c) -> p b c", c=4)
                pf = pgp.tile([1, 256], F32, tag="pg")
                # chunks 0..2 over all 128 partitions
                nc.tensor.matmul(out=pf[:, 0:192], lhsT=ones[:],
                                 rhs=s3[:, :, 0:3], start=True, stop=True)
                # chunk 3 over partitions 0:16 only
                nc.tensor.matmul(out=pf[:, 192:256], lhsT=ones[0:16, :],
                                 rhs=s3[0:16, :, 3:4], start=True, stop=True)
                part = cpool.tile([1, 64], F32, tag="fin_part")
                nc.vector.reduce_sum(
                    out=part[:],
                    in_=pf[:, 0:192].rearrange("p (b c) -> p b c", c=3),
                    axis=AX)
                nc.vector.tensor_add(out=out_t[:], in0=part[:],
                                     in1=pf[:, 192:256])
            den = cpool.tile([1, 64], F32)
            nc.vector.tensor_mul(out=den[:], in0=q2r[:], in1=a2r[:])
            sden = cpool.tile([1, 64], F32)
            nc.scalar.activation(out=sden[:], in_=den[:], func=AF.Sqrt)
            inv = cpool.tile([1, 64], F32)
            nc.vector.reciprocal(out=inv[:], in_=sden[:])
            res = cpool.tile([1, 64], F32)
            nc.vector.tensor_mul(out=res[:], in0=dot[:], in1=inv[:])
            nc.sync.dma_start(out_d.rearrange("(a b) -> a b", a=1), res[:])
    return nc


_BUILT = {}


def get_built():
    if "nc" not in _BUILT:
        nc = bacc.Bacc("TRN2", target_bir_lowering=False, debug=False,
                       num_devices=NCORES)
        build_kernel(nc)
        nc.compile()
        _BUILT["nc"] = nc
    return _BUILT["nc"]


def prep_inputs(question, answer, emb_table, conv_w, conv_b, U):
    f16 = np.float16
    f8 = ml_dtypes.float8_e4m3

    # host tables: C_k[v] = emb[v] @ w[:,:,k]^T, bias folded into C_1,
    # plus a zero sentinel row at index V1 for sequence edges.
    emb32 = emb_table.astype(np.float32)
    ccs = []
    for k in range(3):
        t = emb32 @ conv_w[:, :, k].astype(np.float32).T
        if k == 1:
            t = t + conv_b.astype(np.float32)[None, :]
        cc = np.zeros((VROWS, F), dtype=f16)
        cc[:V1] = t.astype(f16)
        ccs.append(cc)

    # U in fp8, x256, padded to [512, 512], packed so each DoubleRow pair of
    # f-chunks is a contiguous 256-col slice: cols [256*(2j+ip)] hold
    # [U8[f-chunk 2ip, g-chunk j] | U8[f-chunk 2ip+1, g-chunk j]].
    upad = np.zeros((512, 512), dtype=np.float32)
    upad[:F, :F] = U.astype(np.float32) * SC_U8
    u8 = upad.astype(f8)
    uu8 = np.zeros((128, 2048), dtype=f8)
    for j in range(4):
        for ip in range(2):
            for t in range(2):
                i = 2 * ip + t
                uu8[:, 256 * (2 * j + ip) + 128 * t:
                    256 * (2 * j + ip) + 128 * t + 128] = \
                    u8[128 * i:128 * i + 128, 128 * j:128 * j + 128]

    qi = question.astype(np.int64)
    ai = answer.astype(np.int64)
    in_maps = []
    for c in range(NCORES):
        qs = qi[c * BL:(c + 1) * BL]     # [BL, 128]
        as_ = ai[c * BL:(c + 1) * BL]    # [BL, 512]
        # token id at (elem, block, position p): block 0 = question,
        # blocks 1..4 = answer chunks of 128.
        tok = np.zeros((BL, NBLK, 128), dtype=np.int64)
        tok[:, 0, :] = qs
        tok[:, 1:, :] = as_.reshape(BL, 4, 128)
        # shifted index sections; sentinel = V1 (zero row)
        def shifted(seq, shift):  # seq [BL, L] -> [BL, L]
            out = np.full_like(seq, V1)
            if shift == -1:
                out[:, 1:] = seq[:, :-1]
            elif shift == 1:
                out[:, :-1] = seq[:, 1:]
            else:
                out = seq.copy()
            return out
        gidx = np.zeros((128, 3 * BL * NBLK), dtype=np.int32)
        for k in range(3):
            qsh = shifted(qs, k - 1)
            ash = shifted(as_, k - 1)
            toksh = np.zeros((BL, NBLK, 128), dtype=np.int64)
            toksh[:, 0, :] = qsh
            toksh[:, 1:, :] = ash.reshape(BL, 4, 128)
            # cols ordered (elem, block); partition = position in block
            sec = toksh.reshape(BL * NBLK, 128).T.astype(np.int32)
            gidx[:, k * BL * NBLK:(k + 1) * BL * NBLK] = sec
        in_maps.append({
            "cc0": ccs[0], "cc1": ccs[1], "cc2": ccs[2],
            "gidx": gidx, "uu8": uu8,
        })
    return in_maps


def kernel(question, answer, emb_table, conv_w, conv_b, U):
    question = np.asarray(question)
    answer = np.asarray(answer)
    emb_table = np.asarray(emb_table, dtype=np.float32)
    conv_w = np.asarray(conv_w, dtype=np.float32)
    conv_b = np.asarray(conv_b, dtype=np.float32)
    U = np.asarray(U, dtype=np.float32)

    nc = get_built()
    in_maps = prep_inputs(question, answer, emb_table, conv_w, conv_b, U)
    res = bass_utils.run_bass_kernel_spmd(nc, in_maps, core_ids=list(range(NCORES)))
    out = np.concatenate([np.asarray(res.results[c]["out"]).reshape(-1)
                          for c in range(NCORES)])
    return out.astype(np.float32)
